# revision 9
# baseline (speedup 1.0000x reference)
"""CrossScan3D Trainium2 kernel.

Computes, for input x[B=2, C=96, 32, 32, 32] f32, the stack of 12 scans
out[B, 12, C, L=32768]: the 6 axis-order flattenings {ijk, ikj, jki, jik,
kij, kji} of each (b, c) 32^3 volume plus their reversals, in the channel
order of the reference:

    s=0: ijk   s=1: ikj   s=2: rev-ijk   s=3: rev-ikj
    s=4: jki   s=5: jik   s=6: rev-jki   s=7: rev-jik
    s=8: kij   s=9: kji   s=10: rev-kij  s=11: rev-kji

Pure data movement; the 302 MB output write is the roofline. Sharding: the
192 (b, c) volumes split 24 per core across 8 cores (no communication).

Per core, volumes are processed 8 at a time in [128, 2048] f32 SBUF tiles:
partition p = v*32 + a (v in 0..3), free = u*1024 + f (u in 0..1), with
volume = base + 4u + v. Per supergroup the 12 scan layouts are built
on-chip with:
  - DVE 32x32 block transpose (nc.vector.transpose) for partition<->free
    minor swaps ("a <-> innermost axis"),
  - strided copies on the scalar (ACT) engine for free-dim major/minor
    swaps,
  - one stream_shuffle with reversed-partition mask and reversed free AP
    producing the fully reversed volume G (every reversed scan of x is the
    forward scan of G).
Each layout then streams out as one 1 MB DMA on the qSP HWDGE ring (4 KB
runs, 3-dim DRAM AP); input loads ride the SWDGE (gpsimd) ring so all
HWDGE traffic stays on ONE physical ring. Per-core traffic is 3 MiB in +
36 MiB out through the 16 shared SDMA engines; at the ~360 GB/s per-core
HBM share that is a ~113 us floor, and both the cost model (117.7 us
single-shot, DMA engines 97% busy, zero inter-DMA gaps) and the best HW
slope readings (~112-115 us/iter; device drifts to ~121 in slow phases)
sit on it. Measured dead ends kept for the record: loads on the qAct
HWDGE ring never beat this config (~120.6 us floor across 10 runs);
alternating stores across both HWDGE rings cost +13% (~137 us) — HWDGE
ring switching is expensive on real HW though the cost model prices it
as free; SG=12 (1.5 MiB DMAs, [128,3072] tiles) was ~3% slower.
"""

import numpy as np

import concourse.bacc as bacc
import concourse.mybir as mybir
from concourse.tile import TileContext
from concourse.bass_utils import run_bass_kernel_spmd

B = 2
C = 96
D = 32
L = D * D * D            # 32768
NV = B * C               # 192 volumes
NCORES = 8
VPC = NV // NCORES       # 24 volumes per core
SG = 8                   # volumes per supergroup
NSG = VPC // SG          # 3 supergroups per core
F2 = 2 * D * D           # 2048 free elements per partition row

FP32 = mybir.dt.float32

_PROGRAM_CACHE = {}


def _emit(nc, pool, x_in, out, load_engine="gpsimd", store_split=False,
          sg=SG, nsg=NSG):
    u_n = sg // 4          # volumes per partition-group column
    f2 = u_n * D * D       # free elements per partition row
    for h in range(nsg):
        base = h * sg

        def dram_ap(s):
            # DRAM AP in SBUF stream order: (v, a) partition-major, then
            # (u, f) — element (vol = base+4u+v, a, f) of out[s].
            return (
                out[s, base:base + sg]
                .rearrange("(u v) (a f) -> v a u f", u=u_n, a=D)
            )

        A = pool.tile([128, f2], FP32, tag="A")
        getattr(nc, load_engine).dma_start(
            out=A[:],
            in_=x_in[base:base + sg].rearrange("(u v) a j k -> v a u j k", u=u_n),
        )

        def fswap(dst, src):
            # dst[p, u, x, y] = src[p, u, y, x]: swap the two free sub-axes
            nc.scalar.copy(
                out=dst.rearrange("p (u x y) -> p u x y", u=u_n, x=D),
                in_=src.rearrange("p (u y x) -> p u x y", u=u_n, y=D),
            )

        T_ikj = pool.tile([128, f2], FP32, tag="T_ikj")
        fswap(T_ikj, A)
        T_kji = pool.tile([128, f2], FP32, tag="T_kji")
        nc.vector.transpose(out=T_kji[:], in_=A[:])
        T_jki = pool.tile([128, f2], FP32, tag="T_jki")
        nc.vector.transpose(out=T_jki[:], in_=T_ikj[:])
        T_jik = pool.tile([128, f2], FP32, tag="T_jik")
        fswap(T_jik, T_jki)
        T_kij = pool.tile([128, f2], FP32, tag="T_kij")
        fswap(T_kij, T_kji)

        # G = fully reversed volume: G[v, i, j, k] = x[vol, 31-i, 31-j, 31-k]
        G = pool.tile([128, f2], FP32, tag="G")
        nc.vector.stream_shuffle(
            G.rearrange("p (u f) -> p u f", u=u_n),
            A.rearrange("p (u f) -> p u f", u=u_n)[:, :, ::-1],
            list(range(31, -1, -1)),
        )

        G_ikj = pool.tile([128, f2], FP32, tag="G_ikj")
        fswap(G_ikj, G)
        G_kji = pool.tile([128, f2], FP32, tag="G_kji")
        nc.vector.transpose(out=G_kji[:], in_=G[:])
        G_jki = pool.tile([128, f2], FP32, tag="G_jki")
        nc.vector.transpose(out=G_jki[:], in_=G_ikj[:])
        G_jik = pool.tile([128, f2], FP32, tag="G_jik")
        fswap(G_jik, G_jki)
        G_kij = pool.tile([128, f2], FP32, tag="G_kij")
        fswap(G_kij, G_kji)

        def store(s, tile):
            eng = nc.scalar if (store_split and s % 2 == 1) else nc.sync
            eng.dma_start(out=dram_ap(s), in_=tile[:])

        store(0, A)
        store(1, T_ikj)
        store(2, G)
        store(3, G_ikj)
        store(4, T_jki)
        store(5, T_jik)
        store(6, G_jki)
        store(7, G_jik)
        store(8, T_kij)
        store(9, T_kji)
        store(10, G_kij)
        store(11, G_kji)


_BUFS2 = ("A", "G", "T_ikj", "T_jki")


class _Pool:
    """Per-tag tile pools so pipeline-critical tiles get 2 buffers."""

    def __init__(self, tc, bufs2_tags):
        self.tc = tc
        self.bufs2_tags = set(bufs2_tags)
        self.cms = {}
        self.pools = {}

    def __enter__(self):
        return self

    def __exit__(self, *exc):
        for cm in reversed(list(self.cms.values())):
            cm.__exit__(*exc)

    def tile(self, shape, dtype, tag):
        if tag not in self.pools:
            bufs = 2 if tag in self.bufs2_tags else 1
            cm = self.tc.tile_pool(name=f"pool_{tag}", bufs=bufs)
            self.cms[tag] = cm
            self.pools[tag] = cm.__enter__()
        return self.pools[tag].tile(shape, dtype, tag=tag, name=tag)


def build_program(loop_n=None, load_engine="gpsimd", store_split=False, sg=SG):
    """SPMD program per core: x[VPC, 32, 32, 32] -> out[12, VPC, L].

    loop_n wraps the workload in a hardware loop re-executing it loop_n
    times (idempotent writes) — used only for performance measurement.
    """
    nsg = VPC // sg
    nc = bacc.Bacc("TRN2", target_bir_lowering=False)
    x_in = nc.dram_tensor("x", [VPC, D, D, D], FP32, kind="ExternalInput")
    out = nc.dram_tensor("out", [12, VPC, L], FP32, kind="ExternalOutput")

    with TileContext(nc) as tc:
        bufs2 = _BUFS2 if nsg > 1 else ()
        with _Pool(tc, bufs2) as pool:
            kw = dict(load_engine=load_engine, store_split=store_split,
                      sg=sg, nsg=nsg)
            if loop_n:
                with tc.For_i(0, loop_n, 1):
                    _emit(nc, pool, x_in, out, **kw)
            else:
                _emit(nc, pool, x_in, out, **kw)
    nc.compile()
    return nc


def build_timing_program(loop_n, **kw):
    return build_program(loop_n=loop_n, **kw)


def get_program():
    if "nc" not in _PROGRAM_CACHE:
        _PROGRAM_CACHE["nc"] = build_program()
    return _PROGRAM_CACHE["nc"]


def make_in_maps(x: np.ndarray):
    xf = np.ascontiguousarray(x.astype(np.float32, copy=False)).reshape(NV, D, D, D)
    return [
        {"x": np.ascontiguousarray(xf[m * VPC:(m + 1) * VPC])} for m in range(NCORES)
    ]


def assemble(results) -> np.ndarray:
    out = np.empty((B, 12, C, L), np.float32)
    for m in range(NCORES):
        o = np.asarray(results[m]["out"]).reshape(12, VPC, L)
        b, c0 = divmod(m * VPC, C)
        out[b, :, c0:c0 + VPC, :] = o
    return out


def kernel(x: np.ndarray) -> np.ndarray:
    nc = get_program()
    res = run_bass_kernel_spmd(nc, make_in_maps(np.asarray(x)), list(range(NCORES)))
    return assemble(res.results)



# revision 18
# speedup vs baseline: 1.0057x; 1.0057x over previous
"""CrossScan3D Trainium2 kernel.

Computes, for input x[B=2, C=96, 32, 32, 32] f32, the stack of 12 scans
out[B, 12, C, L=32768]: the 6 axis-order flattenings {ijk, ikj, jki, jik,
kij, kji} of each (b, c) 32^3 volume plus their reversals, in the channel
order of the reference:

    s=0: ijk   s=1: ikj   s=2: rev-ijk   s=3: rev-ikj
    s=4: jki   s=5: jik   s=6: rev-jki   s=7: rev-jik
    s=8: kij   s=9: kji   s=10: rev-kij  s=11: rev-kji

Pure data movement; the 302 MB output write is the roofline. Sharding: the
192 (b, c) volumes split 24 per core across 8 cores (no communication).

Per core, volumes are processed 8 at a time in [128, 2048] f32 SBUF tiles:
partition p = v*32 + a (v in 0..3), free = u*1024 + f (u in 0..1), with
volume = base + 4u + v. Per supergroup the 12 scan layouts are built
on-chip with:
  - DVE 32x32 block transpose (nc.vector.transpose) for partition<->free
    minor swaps ("a <-> innermost axis"),
  - strided copies on the scalar (ACT) engine for free-dim major/minor
    swaps,
  - one stream_shuffle with reversed-partition mask and reversed free AP
    producing the fully reversed volume G (every reversed scan of x is the
    forward scan of G).
Each layout then streams out as one 1 MB DMA on the qSP HWDGE ring (4 KB
runs, 3-dim DRAM AP); input loads ride the SWDGE (gpsimd) ring so all
HWDGE traffic stays on ONE physical ring. Per-core traffic is 3 MiB in +
36 MiB out through the 16 shared SDMA engines; at the ~360 GB/s per-core
HBM share that is a ~113 us floor, and both the cost model (117.7 us
single-shot, DMA engines 97% busy, zero inter-DMA gaps) and the best HW
slope readings (~112-115 us/iter; device drifts to ~121 in slow phases)
sit on it. Measured dead ends kept for the record: loads on the qAct
HWDGE ring never beat this config (~120.6 us floor across 10 runs);
alternating stores across both HWDGE rings cost +13% (~137 us) — HWDGE
ring switching is expensive on real HW though the cost model prices it
as free; SG=12 (1.5 MiB DMAs, [128,3072] tiles) was ~3% slower;
chunking the first load to start the SDMA engines sooner (ramp_split)
models +485 ns worse — SWDGE's ~1.8 us dispatch pipeline, not
descriptor count, gates the first transfer, and chunking adds a 663 ns
descriptor-generation bubble. Routing only the first load over the qSP
HWDGE ring (first_load_sync) starts the first transfer 450 ns earlier
(model 117.2 us) with no steady-state change.
"""

import numpy as np

import concourse.bacc as bacc
import concourse.mybir as mybir
from concourse.tile import TileContext
from concourse.bass_utils import run_bass_kernel_spmd

B = 2
C = 96
D = 32
L = D * D * D            # 32768
NV = B * C               # 192 volumes
NCORES = 8
VPC = NV // NCORES       # 24 volumes per core
SG = 8                   # volumes per supergroup
NSG = VPC // SG          # 3 supergroups per core
F2 = 2 * D * D           # 2048 free elements per partition row

FP32 = mybir.dt.float32

_PROGRAM_CACHE = {}


def _emit(nc, pool, x_in, out, load_engine="gpsimd", store_split=False,
          sg=SG, nsg=NSG, ramp_split=False, first_load_sync=False):
    u_n = sg // 4          # volumes per partition-group column
    f2 = u_n * D * D       # free elements per partition row
    FD = D * D             # 1024 free elements per volume-column
    for h in range(nsg):
        base = h * sg

        def dram_ap(s):
            # DRAM AP in SBUF stream order: (v, a) partition-major, then
            # (u, f) — element (vol = base+4u+v, a, f) of out[s].
            return (
                out[s, base:base + sg]
                .rearrange("(u v) (a f) -> v a u f", u=u_n, a=D)
            )

        A = pool.tile([128, f2], FP32, tag="A")
        eng_name = "sync" if (first_load_sync and h == 0) else load_engine
        load = getattr(nc, eng_name).dma_start
        if ramp_split and h == 0:
            # Chunked first load + matching store-0 chunks: the first 32-
            # descriptor chunk puts the SDMA engines to work ~1 us sooner
            # in single-shot execution (compute and stores s>=1 need the
            # full tile, so only the s=0 identity store is chunked).
            load(out=A[0:32, 0:FD],
                 in_=x_in[base].rearrange("a j k -> a (j k)"))
            load(out=A[32:128, 0:FD],
                 in_=x_in[base + 1:base + 4].rearrange("v a j k -> (v a) (j k)"))
            for u in range(1, u_n):
                load(out=A[:, u * FD:(u + 1) * FD],
                     in_=x_in[base + 4 * u:base + 4 * (u + 1)]
                     .rearrange("v a j k -> (v a) (j k)"))
        else:
            load(out=A[:],
                 in_=x_in[base:base + sg]
                 .rearrange("(u v) a j k -> v a u j k", u=u_n))

        def fswap(dst, src):
            # dst[p, u, x, y] = src[p, u, y, x]: swap the two free sub-axes
            nc.scalar.copy(
                out=dst.rearrange("p (u x y) -> p u x y", u=u_n, x=D),
                in_=src.rearrange("p (u y x) -> p u x y", u=u_n, y=D),
            )

        T_ikj = pool.tile([128, f2], FP32, tag="T_ikj")
        fswap(T_ikj, A)
        T_kji = pool.tile([128, f2], FP32, tag="T_kji")
        nc.vector.transpose(out=T_kji[:], in_=A[:])
        T_jki = pool.tile([128, f2], FP32, tag="T_jki")
        nc.vector.transpose(out=T_jki[:], in_=T_ikj[:])
        T_jik = pool.tile([128, f2], FP32, tag="T_jik")
        fswap(T_jik, T_jki)
        T_kij = pool.tile([128, f2], FP32, tag="T_kij")
        fswap(T_kij, T_kji)

        # G = fully reversed volume: G[v, i, j, k] = x[vol, 31-i, 31-j, 31-k]
        G = pool.tile([128, f2], FP32, tag="G")
        nc.vector.stream_shuffle(
            G.rearrange("p (u f) -> p u f", u=u_n),
            A.rearrange("p (u f) -> p u f", u=u_n)[:, :, ::-1],
            list(range(31, -1, -1)),
        )

        G_ikj = pool.tile([128, f2], FP32, tag="G_ikj")
        fswap(G_ikj, G)
        G_kji = pool.tile([128, f2], FP32, tag="G_kji")
        nc.vector.transpose(out=G_kji[:], in_=G[:])
        G_jki = pool.tile([128, f2], FP32, tag="G_jki")
        nc.vector.transpose(out=G_jki[:], in_=G_ikj[:])
        G_jik = pool.tile([128, f2], FP32, tag="G_jik")
        fswap(G_jik, G_jki)
        G_kij = pool.tile([128, f2], FP32, tag="G_kij")
        fswap(G_kij, G_kji)

        def store(s, tile):
            eng = nc.scalar if (store_split and s % 2 == 1) else nc.sync
            eng.dma_start(out=dram_ap(s), in_=tile[:])

        if ramp_split and h == 0:
            nc.sync.dma_start(
                out=out[0, base].rearrange("(a f) -> a f", a=D),
                in_=A[0:32, 0:FD])
            nc.sync.dma_start(
                out=out[0, base + 1:base + 4].rearrange("v (a f) -> (v a) f", a=D),
                in_=A[32:128, 0:FD])
            for u in range(1, u_n):
                nc.sync.dma_start(
                    out=out[0, base + 4 * u:base + 4 * (u + 1)]
                    .rearrange("v (a f) -> (v a) f", a=D),
                    in_=A[:, u * FD:(u + 1) * FD])
        else:
            store(0, A)
        store(1, T_ikj)
        store(2, G)
        store(3, G_ikj)
        store(4, T_jki)
        store(5, T_jik)
        store(6, G_jki)
        store(7, G_jik)
        store(8, T_kij)
        store(9, T_kji)
        store(10, G_kij)
        store(11, G_kji)


_BUFS2 = ("A", "G", "T_ikj", "T_jki")


class _Pool:
    """Per-tag tile pools so pipeline-critical tiles get 2 buffers."""

    def __init__(self, tc, bufs2_tags):
        self.tc = tc
        self.bufs2_tags = set(bufs2_tags)
        self.cms = {}
        self.pools = {}

    def __enter__(self):
        return self

    def __exit__(self, *exc):
        for cm in reversed(list(self.cms.values())):
            cm.__exit__(*exc)

    def tile(self, shape, dtype, tag):
        if tag not in self.pools:
            bufs = 2 if tag in self.bufs2_tags else 1
            cm = self.tc.tile_pool(name=f"pool_{tag}", bufs=bufs)
            self.cms[tag] = cm
            self.pools[tag] = cm.__enter__()
        return self.pools[tag].tile(shape, dtype, tag=tag, name=tag)


def build_program(loop_n=None, load_engine="gpsimd", store_split=False, sg=SG,
                  ramp_split=False, first_load_sync=True):
    """SPMD program per core: x[VPC, 32, 32, 32] -> out[12, VPC, L].

    loop_n wraps the workload in a hardware loop re-executing it loop_n
    times (idempotent writes) — used only for performance measurement.
    """
    nsg = VPC // sg
    nc = bacc.Bacc("TRN2", target_bir_lowering=False)
    x_in = nc.dram_tensor("x", [VPC, D, D, D], FP32, kind="ExternalInput")
    out = nc.dram_tensor("out", [12, VPC, L], FP32, kind="ExternalOutput")

    with TileContext(nc) as tc:
        bufs2 = _BUFS2 if nsg > 1 else ()
        with _Pool(tc, bufs2) as pool:
            kw = dict(load_engine=load_engine, store_split=store_split,
                      sg=sg, nsg=nsg, ramp_split=ramp_split,
                      first_load_sync=first_load_sync)
            if loop_n:
                with tc.For_i(0, loop_n, 1):
                    _emit(nc, pool, x_in, out, **kw)
            else:
                _emit(nc, pool, x_in, out, **kw)
    nc.compile()
    return nc


def build_timing_program(loop_n, **kw):
    return build_program(loop_n=loop_n, **kw)


def get_program():
    if "nc" not in _PROGRAM_CACHE:
        _PROGRAM_CACHE["nc"] = build_program()
    return _PROGRAM_CACHE["nc"]


def make_in_maps(x: np.ndarray):
    xf = np.ascontiguousarray(x.astype(np.float32, copy=False)).reshape(NV, D, D, D)
    return [
        {"x": np.ascontiguousarray(xf[m * VPC:(m + 1) * VPC])} for m in range(NCORES)
    ]


def assemble(results) -> np.ndarray:
    out = np.empty((B, 12, C, L), np.float32)
    for m in range(NCORES):
        o = np.asarray(results[m]["out"]).reshape(12, VPC, L)
        b, c0 = divmod(m * VPC, C)
        out[b, :, c0:c0 + VPC, :] = o
    return out


def kernel(x: np.ndarray) -> np.ndarray:
    nc = get_program()
    res = run_bass_kernel_spmd(nc, make_in_maps(np.asarray(x)), list(range(NCORES)))
    return assemble(res.results)

